# revision 1
# baseline (speedup 1.0000x reference)
"""GQA kernel for TRN2, 8-way tensor-parallel by KV head.

Per core i: KV head i, Q heads 4i..4i+3.
  - Projections computed transposed (Q^T, K^T, V^T) from host-transposed x^T,
    in fp32r (full PE rate, fp32-equivalent accuracy, operands DMA-fed).
  - Scores computed transposed: S^T[k, q] = K Q^T -> softmax along free dim is
    avoided entirely: exp (no max-subtract; logits are O(5)), denominator via a
    ones-column appended to V in the AV matmul (partition-dim sum on PE).
  - P^T, V, Y^T in bf16 (engine-produced matmul operands can't be fp32r).
  - Causal masking: memset zeros for fully-invalid column blocks + triangular
    mask multiply on the diagonal 128x128 block.
  - Out projection per core uses the local 256 rows of Wo; host sums the 8
    partial products (no on-device collective needed).
"""

import sys

for p in ("/opt/trn_rl_repo", "/root/.axon_site/_ro/trn_rl_repo"):
    if p not in sys.path:
        sys.path.insert(0, p)

import numpy as np
import ml_dtypes
from contextlib import ExitStack

import concourse.bacc as bacc
import concourse.mybir as mybir
import concourse.tile as tile

F32 = mybir.dt.float32
F32R = mybir.dt.float32r
BF16 = mybir.dt.bfloat16
BF16_NP = ml_dtypes.bfloat16

N_Q_LOCAL = 4  # q heads per core
D = 64
ROPE_BASE = 10000.0
QC = 256  # projection chunk (t columns)
AC = 512  # attention chunk (q columns)


def build_nc(C, T, B):
    CT = C // 128  # contraction tiles
    NCH = T // AC  # attention chunks per batch
    NQ2 = AC // QC
    NCO = C // 512  # out-proj column chunks
    KT_PER_B = T // 128

    nc = bacc.Bacc("TRN2", target_bir_lowering=False, debug=False)

    xT = nc.dram_tensor("xT", [C, B * T], F32R, kind="ExternalInput")
    wq = nc.dram_tensor("wq", [C, 256], F32R, kind="ExternalInput")
    wkv = nc.dram_tensor("wkv", [C, 128], F32R, kind="ExternalInput")
    wo = nc.dram_tensor("wo", [256, C], BF16, kind="ExternalInput")
    rqc = nc.dram_tensor("rqc", [128, T], F32, kind="ExternalInput")
    rqs = nc.dram_tensor("rqs", [128, T], F32, kind="ExternalInput")
    rkc = nc.dram_tensor("rkc", [64, T], F32, kind="ExternalInput")
    rks = nc.dram_tensor("rks", [64, T], F32, kind="ExternalInput")
    ident = nc.dram_tensor("ident", [64, 64], F32, kind="ExternalInput")
    tri = nc.dram_tensor("tri", [128, 128], BF16, kind="ExternalInput")
    out = nc.dram_tensor("out", [B * T, C], F32, kind="ExternalOutput")

    with tile.TileContext(nc) as tc, ExitStack() as ctx:
        # PSUM pools: 3 + 3 + 2 = 8 banks
        pp = ctx.enter_context(tc.tile_pool(name="pp", bufs=2, space="PSUM"))
        pss = ctx.enter_context(tc.tile_pool(name="pss", bufs=2, space="PSUM"))
        psy = ctx.enter_context(tc.tile_pool(name="psy", bufs=2, space="PSUM"))

        cst = ctx.enter_context(tc.tile_pool(name="cst", bufs=1))
        xcp = ctx.enter_context(tc.tile_pool(name="xcp", bufs=2))
        ktp = ctx.enter_context(tc.tile_pool(name="ktp", bufs=2))
        vtsp = ctx.enter_context(tc.tile_pool(name="vtsp", bufs=2))
        vpp = ctx.enter_context(tc.tile_pool(name="vpp", bufs=2))
        qtp = ctx.enter_context(tc.tile_pool(name="qtp", bufs=10))
        tmpp = ctx.enter_context(tc.tile_pool(name="tmpp", bufs=6))
        ptp = ctx.enter_context(tc.tile_pool(name="ptp", bufs=6))
        ytp_ = ctx.enter_context(tc.tile_pool(name="ytp", bufs=6))
        dnp = ctx.enter_context(tc.tile_pool(name="dnp", bufs=3))
        bcp = ctx.enter_context(tc.tile_pool(name="bcp", bufs=3))
        osp = ctx.enter_context(tc.tile_pool(name="osp", bufs=6))

        WQ = cst.tile([128, CT * 256], F32R, tag="WQ")
        WKV = cst.tile([128, CT * 128], F32R, tag="WKV")
        WO = [cst.tile([128, C], BF16, tag=f"WO{i}", name=f"WO{i}") for i in range(2)]
        RQC = cst.tile([128, T], F32, tag="RQC")
        RQS = cst.tile([128, T], F32, tag="RQS")
        RKC = cst.tile([64, T], F32, tag="RKC")
        RKS = cst.tile([64, T], F32, tag="RKS")
        ID = cst.tile([64, 64], F32, tag="ID")
        TRI = cst.tile([128, 128], BF16, tag="TRI")

        for ct in range(CT):
            nc.sync.dma_start(WQ[:, ct * 256:(ct + 1) * 256], wq[ct * 128:(ct + 1) * 128, :])
            nc.sync.dma_start(WKV[:, ct * 128:(ct + 1) * 128], wkv[ct * 128:(ct + 1) * 128, :])
        nc.sync.dma_start(WO[0][:], wo[0:128, :])
        nc.sync.dma_start(WO[1][:], wo[128:256, :])
        nc.sync.dma_start(RQC[:], rqc[:])
        nc.sync.dma_start(RQS[:], rqs[:])
        nc.sync.dma_start(RKC[:], rkc[:])
        nc.sync.dma_start(RKS[:], rks[:])
        nc.sync.dma_start(ID[:], ident[:])
        nc.sync.dma_start(TRI[:], tri[:])

        def rope(src_ps, r0, cosT, sinT, dst_ap, w, tw0):
            """dst[0:64, :w] (bf16) = RoPE(src_ps[r0:r0+64, :w]) using tables at
            rows r0 (cos) / shifted-sign sin; tw0 = t-column offset in tables."""
            tco = tmpp.tile([64, QC], F32, tag="tco")
            tsi = tmpp.tile([64, QC], F32, tag="tsi")
            nc.vector.tensor_mul(tco[0:64, 0:w], src_ps[r0:r0 + 64, 0:w], cosT[r0:r0 + 64, tw0:tw0 + w])
            nc.vector.tensor_mul(tsi[0:32, 0:w], src_ps[r0 + 32:r0 + 64, 0:w], sinT[r0 + 32:r0 + 64, tw0:tw0 + w])
            nc.vector.tensor_mul(tsi[32:64, 0:w], src_ps[r0:r0 + 32, 0:w], sinT[r0:r0 + 32, tw0:tw0 + w])
            nc.vector.tensor_add(dst_ap, tco[0:64, 0:w], tsi[0:64, 0:w])

        def rope_pair(src_ps, cosT, sinT, dst0, dst1, w, tw0):
            """RoPE both heads of a [128, w] psum pair in 6 DVE ops.
            dst0/dst1 are [64, w] bf16 APs for heads at rows 0:64 / 64:128."""
            tco = tmpp.tile([128, QC], F32, tag="tco2")
            tsi = tmpp.tile([128, QC], F32, tag="tsi2")
            nc.vector.tensor_mul(tco[0:128, 0:w], src_ps[0:128, 0:w], cosT[0:128, tw0:tw0 + w])
            for b0 in (0, 64):
                nc.vector.tensor_mul(tsi[b0:b0 + 32, 0:w], src_ps[b0 + 32:b0 + 64, 0:w], sinT[b0 + 32:b0 + 64, tw0:tw0 + w])
                nc.vector.tensor_mul(tsi[b0 + 32:b0 + 64, 0:w], src_ps[b0:b0 + 32, 0:w], sinT[b0:b0 + 32, tw0:tw0 + w])
            nc.vector.tensor_add(dst0, tco[0:64, 0:w], tsi[0:64, 0:w])
            nc.vector.tensor_add(dst1, tco[64:128, 0:w], tsi[64:128, 0:w])

        for b in range(B):
            KT = ktp.tile([64, T], BF16, tag="KT")
            VTS = vtsp.tile([64, T], F32, tag="VTS")
            VP = vpp.tile([128, KT_PER_B * 65], BF16, tag="VP")

            for ch in range(NCH):
                QT = [qtp.tile([64, AC], BF16, tag="QT", name=f"QT{b}_{ch}_{_h}") for _h in range(N_Q_LOCAL)]
                for q2 in range(NQ2):
                    tcol = ch * AC + q2 * QC
                    col = b * T + tcol
                    XC = xcp.tile([128, CT * QC], F32R, tag="XC")
                    for ct in range(CT):
                        nc.sync.dma_start(
                            XC[:, ct * QC:(ct + 1) * QC],
                            xT[ct * 128:(ct + 1) * 128, col:col + QC],
                        )
                    # KV projection
                    PKV = pp.tile([128, QC], F32, tag="pp")
                    for ct in range(CT):
                        nc.tensor.matmul(
                            PKV[:], WKV[:, ct * 128:(ct + 1) * 128],
                            XC[:, ct * QC:(ct + 1) * QC],
                            start=(ct == 0), stop=(ct == CT - 1),
                        )
                    # K rope -> KT
                    rope(PKV, 0, RKC, RKS, KT[0:64, tcol:tcol + QC], QC, tcol)
                    # V staging (cross-partition via DMA) + transpose + ones col
                    nc.scalar.copy(VTS[0:64, tcol:tcol + QC], PKV[64:128, :])
                    for k2 in range(QC // 128):
                        kg = (tcol + k2 * 128) // 128
                        PT_ps = pp.tile([128, 64], F32, tag="pp")
                        nc.tensor.transpose(PT_ps[:], VTS[0:64, kg * 128:(kg + 1) * 128], ID[:])
                        nc.vector.tensor_copy(VP[:, kg * 65:kg * 65 + 64], PT_ps[:])
                        nc.vector.memset(VP[:, kg * 65 + 64:kg * 65 + 65], 1.0)
                    # Q projection per head pair
                    for hp in range(2):
                        PQ = pp.tile([128, QC], F32, tag="pp")
                        for ct in range(CT):
                            nc.tensor.matmul(
                                PQ[:], WQ[:, ct * 256 + hp * 128:ct * 256 + hp * 128 + 128],
                                XC[:, ct * QC:(ct + 1) * QC],
                                start=(ct == 0), stop=(ct == CT - 1),
                            )
                        for hl in range(2):
                            h = 2 * hp + hl
                            rope(PQ, hl * 64, RQC, RQS,
                                 QT[h][0:64, q2 * QC:q2 * QC + QC], QC, tcol)

                # attention for chunk ch
                nkt = (ch + 1) * (AC // 128)
                YTC = [ytp_.tile([128, AC], BF16, tag="YTC", name=f"YTC{b}_{ch}_{_i}") for _i in range(2)]
                for h in range(N_Q_LOCAL):
                    YPS = psy.tile([65, AC], F32, tag="psy")
                    for ki in range(nkt):
                        S = pss.tile([128, AC], F32, tag="pss")
                        nc.tensor.matmul(
                            S[:], KT[0:64, ki * 128:(ki + 1) * 128], QT[h][0:64, :],
                            start=True, stop=True,
                        )
                        P = ptp.tile([128, AC], BF16, tag="P")
                        lm = ki - ch * (AC // 128)
                        if lm >= 0:  # in-chunk: diagonal masking
                            if lm > 0:
                                nc.vector.memset(P[:, 0:lm * 128], 0.0)
                            nc.scalar.activation(
                                P[:, lm * 128:AC], S[:, lm * 128:AC],
                                mybir.ActivationFunctionType.Exp,
                            )
                            nc.vector.tensor_mul(
                                P[:, lm * 128:lm * 128 + 128],
                                P[:, lm * 128:lm * 128 + 128], TRI[:],
                            )
                        else:
                            nc.scalar.activation(P[:], S[:], mybir.ActivationFunctionType.Exp)
                        nc.tensor.matmul(
                            YPS[:], VP[:, ki * 65:(ki + 1) * 65], P[:],
                            start=(ki == 0), stop=(ki == nkt - 1),
                        )
                    # normalize: denom row -> recip (plain, direct from PSUM) -> broadcast -> mul
                    RC = dnp.tile([1, AC], F32, tag="RC")
                    nc.vector.reciprocal(RC[0:1, :], YPS[64:65, :])
                    BC = bcp.tile([64, AC], F32, tag="BC")
                    nc.gpsimd.partition_broadcast(BC[:], RC[0:1, :])
                    nc.vector.tensor_mul(
                        YTC[h // 2][(h % 2) * 64:(h % 2) * 64 + 64, :],
                        YPS[0:64, :], BC[:],
                    )
                # out projection for this chunk
                for tt in range(AC // 128):
                    trow = b * T + ch * AC + tt * 128
                    for co in range(NCO):
                        PO = pss.tile([128, 512], F32, tag="po")
                        for cl in range(2):
                            nc.tensor.matmul(
                                PO[:], YTC[cl][:, tt * 128:(tt + 1) * 128],
                                WO[cl][:, co * 512:(co + 1) * 512],
                                start=(cl == 0), stop=(cl == 1),
                            )
                        OS = osp.tile([128, 512], F32, tag="OS")
                        nc.scalar.copy(OS[:], PO[:])
                        nc.sync.dma_start(out[trow:trow + 128, co * 512:(co + 1) * 512], OS[:])

    nc.compile()
    return nc


def rope_tables(T, scale):
    inv = 1.0 / (ROPE_BASE ** (np.arange(0, D, 2, dtype=np.float32) / D))
    t = np.arange(T, dtype=np.float32)
    freqs = np.outer(t, inv)  # [T, 32]
    emb = np.concatenate([freqs, freqs], -1)  # [T, 64]
    cos = np.cos(emb).T.astype(np.float32) * scale  # [64, T]
    sin = np.sin(emb).T.astype(np.float32) * scale
    sinX = np.empty((64, T), np.float32)
    sinX[0:32] = sin[32:64]
    sinX[32:64] = -sin[0:32]
    return np.ascontiguousarray(cos), np.ascontiguousarray(sinX)


def make_inputs(x, Wq, Wk, Wv, Wo):
    B, T, C = x.shape
    xT = np.ascontiguousarray(x.reshape(B * T, C).T)
    qc, qs = rope_tables(T, 1.0 / np.sqrt(D).astype(np.float32))
    kc, ks = rope_tables(T, 1.0)
    rqc = np.concatenate([qc, qc], 0)
    rqs = np.concatenate([qs, qs], 0)
    common = {
        "xT": xT,
        "rqc": rqc, "rqs": rqs, "rkc": kc, "rks": ks,
        "ident": np.eye(64, dtype=np.float32),
        "tri": np.triu(np.ones((128, 128))).astype(BF16_NP),
    }
    in_maps = []
    for i in range(8):
        m = dict(common)
        m["wq"] = np.ascontiguousarray(Wq[:, i * 256:(i + 1) * 256])
        m["wkv"] = np.ascontiguousarray(
            np.concatenate([Wk[:, i * 64:(i + 1) * 64], Wv[:, i * 64:(i + 1) * 64]], 1))
        m["wo"] = np.ascontiguousarray(Wo[i * 256:(i + 1) * 256, :]).astype(BF16_NP)
        in_maps.append(m)
    return in_maps


_NC_CACHE = {}


def _get_nc(C, T, B):
    key = (C, T, B)
    if key not in _NC_CACHE:
        _NC_CACHE[key] = build_nc(C, T, B)
    return _NC_CACHE[key]


def run(x, Wq, Wk, Wv, Wo, trace=False):
    from concourse.bass_utils import run_bass_kernel_spmd

    B, T, C = x.shape
    nc = _get_nc(C, T, B)
    in_maps = make_inputs(x, Wq, Wk, Wv, Wo)
    try:
        res = run_bass_kernel_spmd(nc, in_maps, list(range(8)), trace=trace)
    except (ImportError, ModuleNotFoundError):
        res = run_bass_kernel_spmd(nc, in_maps, list(range(8)), trace=False)
    acc = res.results[0]["out"].astype(np.float32)
    for i in range(1, 8):
        acc = acc + res.results[i]["out"].astype(np.float32)
    return acc.reshape(B, T, C), res


def kernel(x, Wq, Wk, Wv, Wo):
    out, _ = run(x, Wq, Wk, Wv, Wo, trace=False)
    return out

